# revision 29
# baseline (speedup 1.0000x reference)
"""Trainium2 Bass kernel for SoftMoE (LayerNorm + cosine routing + per-expert MLP).

Sharding: pure data-parallel over batch B=8 -> one batch element per NeuronCore.
No collectives. Each core computes its full (N, D) output slice.

Math notes (per core, x is (N, D)):
  x_ln = LN(x) * gamma + beta
  x_n  = x_ln * t[n],   t[n] = scale / ||x_ln[n]||
  logitsT[es, n] = minv[es] * (mu_q.T @ xq.T)  (mu_q = 32*mu fp8, xq = 32*x_n
                   fp8, minv = 1/(1024*||mu||) folds both 32x scales)
  E = exp(logits); dispatch = E/sd[es] (softmax over n); combine = E/sc[n]
  slot_raw[d,es] = sum_n x_ln[n,d] E[n,es] = A[d] + sum_n x_ln (E-1)
  h  = gelu(sdinv[es] * slot_raw @ W1 + b1);  so = h @ W2 + b2
  out[n] = (A2[d] + sum_es (E-1) so[es,d]) / sc[n],  A2 = colsum(so)
The (E-1)-centered forms let dispatch/combine run as fp8 DoubleRow matmuls
(E~1 would vanish under fp8 quantization); the rank-1 A/A2 corrections are
cheap K=1 bf16 matmuls. sd comes from the exp eviction's accum_out; sc from
ones-lhsT rowsum matmuls over E during the logits phase.

Staging: x/W1/W2 bf16, mu fp8e4 (x32 pre-scale) -- host-side casts. MLP stays
bf16 (weight quantization error would be correlated across slots and not
average out in the combine). The per-expert MLPs are software-pipelined into
the NEXT dispatch chunk's matmul stream so the 4MB/expert weight DMA is
consumed evenly instead of in bursts.
"""

import numpy as np
from contextlib import ExitStack

import concourse.bass as bass
import concourse.tile as tile
from concourse import bacc
from concourse import mybir
from concourse.masks import make_identity

FP32 = mybir.dt.float32
BF16 = mybir.dt.bfloat16
FP8 = mybir.dt.float8e4
DR = mybir.MatmulPerfMode.DoubleRow
AF = mybir.ActivationFunctionType
ALU = mybir.AluOpType
AX = mybir.AxisListType

P = 128
LN_EPS = 1e-5
MUS = 32.0    # host-side mu pre-scale (fp8 range centering)
XQS = 32.0    # on-device x_n pre-scale for fp8 logits / dispatch lhsT
EDS = 64.0    # (E-1) pre-scale for fp8 dispatch/combine moving operands
SOS = 16.0    # slot_out pre-scale for the fp8 combine rhs

SIM_SAFE_GELU = False


def _bcast_ap(handle, p, free):
    """AP reading a 1-D DRAM tensor broadcast across p partitions."""
    return bass.AP(tensor=handle, offset=0, ap=[[0, p], [1, free]])


def build_softmoe(N, D, E, S, H, *, apply_gamma_beta=True, apply_b1=True,
                  apply_b2=True):
    assert S == P
    ES = E * S
    NT, KD, NE, QH = N // P, D // P, ES // P, H // P
    CN = min(512, N); JN = N // CN       # n-chunks
    CE = min(512, ES); JE = ES // CE     # es-chunks
    CD = min(512, D); JD = D // CD       # d-chunks
    EPC = CE // P                        # experts per es-chunk
    TPC = CN // P                        # P-tiles per n-chunk
    MUB = 4                              # mu k-tiles per load batch

    nc = bacc.Bacc(None, target_bir_lowering=False, debug=False)

    x_h = nc.dram_tensor("x", [N, D], BF16, kind="ExternalInput")
    g_h = nc.dram_tensor("gamma", [D], FP32, kind="ExternalInput")
    be_h = nc.dram_tensor("beta", [D], FP32, kind="ExternalInput")
    mu_h = nc.dram_tensor("mu", [D, E, S], FP8, kind="ExternalInput")
    sc_h = nc.dram_tensor("scale", [1], FP32, kind="ExternalInput")
    w1_h = nc.dram_tensor("W1", [E, D, H], BF16, kind="ExternalInput")
    b1_h = nc.dram_tensor("b1", [E, H], FP32, kind="ExternalInput")
    w2_h = nc.dram_tensor("W2", [E, H, D], BF16, kind="ExternalInput")
    b2_h = nc.dram_tensor("b2", [E, D], FP32, kind="ExternalInput")
    out_h = nc.dram_tensor("out", [N, D], FP32, kind="ExternalOutput")

    et_d = nc.dram_tensor("et_scr", [ES, N], BF16)

    with tile.TileContext(nc, pool_alloc_mode="queue") as tc, ExitStack() as ctx:
        small = ctx.enter_context(tc.tile_pool(name="small", bufs=1))
        psum = ctx.enter_context(tc.tile_pool(name="psum", bufs=6, space="PSUM"))

        ones_b = small.tile([P, 1], BF16, tag="ones_b")
        nc.vector.memset(ones_b, 1.0)
        ones_row = small.tile([1, CE], BF16, tag="ones_row")
        nc.vector.memset(ones_row, 1.0)
        s_bc = small.tile([P, 1], FP32, tag="s_bc")
        nc.gpsimd.dma_start(out=s_bc, in_=_bcast_ap(sc_h, P, 1))
        tinv = small.tile([P, NT], FP32, tag="tinv")
        minv = small.tile([P, NE], FP32, tag="minv")
        sd = small.tile([P, NE], FP32, tag="sd")
        sdinv = small.tile([P, NE], FP32, tag="sdinv")
        scv = small.tile([P, NT], FP32, tag="scv")
        scinvq = small.tile([P, NT], FP32, tag="scinvq")
        ident_b = small.tile([P, P], BF16, tag="ident_b")
        make_identity(nc, ident_b)
        ident_f = small.tile([P, P], FP32, tag="ident_f")
        make_identity(nc, ident_f)
        if apply_gamma_beta:
            gm_bc = small.tile([P, D], FP32, tag="gm_bc")
            nc.gpsimd.dma_start(out=gm_bc, in_=_bcast_ap(g_h, P, D))
            bt_bc = small.tile([P, D], FP32, tag="bt_bc")
            nc.gpsimd.dma_start(out=bt_bc, in_=_bcast_ap(be_h, P, D))

        rows = ctx.enter_context(tc.tile_pool(name="rows", bufs=1))
        n2c = small.tile([P, NE], FP32, tag="n2c")
        A_sb = rows.tile([1, D], BF16, tag="A_sb")
        Aacc = rows.tile([1, D], FP32, tag="Aacc")
        nc.vector.memset(Aacc, 0.0)
        A2acc = rows.tile([1, D], FP32, tag="A2acc")
        nc.vector.memset(A2acc, 0.0)
        A2_sb = rows.tile([1, D], BF16, tag="A2_sb")
        # fp8 32*x_n in [n-part, n-tile, d] layout: resident through P3 as
        # the dispatch DoubleRow lhsT and the transpose source for logits
        xqp = ctx.enter_context(tc.tile_pool(name="xq_pool", bufs=1))
        xq = xqp.tile([P, NT, D], FP8, tag="xq")
        mub_ctx = ExitStack()
        mubp = mub_ctx.enter_context(tc.tile_pool(name="mub_pool", bufs=1))
        mubc = [mubp.tile([P, MUB, ES], FP8, tag=f"mubc{b}", name=f"mubc{b}")
                for b in range(KD // MUB)]

        # ------------- fused P1+P2: LN -> x_nT (PE transpose) -> logits ------
        sinv_bc = small.tile([P, 1], FP32, tag="sinv_bc")
        nc.vector.reciprocal(out=sinv_bc[:], in_=s_bc[:])
        sdall = small.tile([P, NE * JN], FP32, tag="sdall")
        ssq_t = small.tile([P, NT], FP32, tag="ssq_t")
        with tc.tile_pool(name="p1", bufs=2) as p1, \
                tc.tile_pool(name="p1n", bufs=3) as p1n, \
                tc.tile_pool(name="p1s", bufs=8) as p1s, \
                tc.tile_pool(name="rows_f", bufs=1) as rows_f, \
                tc.tile_pool(name="p2a", bufs=2) as p2a, \
                tc.tile_pool(name="xnT_pool", bufs=3) as xntp, \
                tc.tile_pool(name="p2b", bufs=6) as p2b:
            arow_sb = rows_f.tile([1, ES], FP32, tag="arow_sb")
            sc_sb = rows_f.tile([1, N], FP32, tag="sc_sb")
            for b in range(KD // MUB):
                nc.scalar.dma_start(
                    out=mubc[b][:],
                    in_=mu_h[b * MUB * P:(b + 1) * MUB * P].rearrange(
                        "(k p) e s -> p k (e s)", p=P))
            # mu column sq-norms as rows (cheap 1-col-LDW rowsum matmuls),
            # then 16 tiny PE transposes back to column layout.
            arow = [psum.tile([1, CE], FP32, tag="mmps", name=f"arow{c}")
                    for c in range(JE)]
            for k in range(KD):
                msq = p2a.tile([P, ES], BF16, tag="msq")
                nc.vector.tensor_mul(msq[:], mubc[k // MUB][:, k % MUB, :],
                                     mubc[k // MUB][:, k % MUB, :])
                for c in range(JE):
                    nc.tensor.matmul(arow[c][:], ones_b[:],
                                     msq[:, c * CE:(c + 1) * CE],
                                     start=(k == 0), stop=(k == KD - 1),
                                     skip_group_check=True)
            for c in range(JE):
                nc.scalar.copy(out=arow_sb[:, c * CE:(c + 1) * CE],
                               in_=arow[c][:])
            for e in range(NE):
                pstn = psum.tile([P, 1], FP32, tag="pst", name=f"pstn{e}",
                                 bufs=2)
                nc.tensor.transpose(pstn[:], arow_sb[:1, e * P:(e + 1) * P],
                                    ident_f[:1, :1])
                nc.vector.tensor_copy(out=n2c[:, e:e + 1], in_=pstn[:])
            sqn = small.tile([P, NE], FP32, tag="sqn")
            nc.scalar.activation(out=sqn[:], in_=n2c[:], func=AF.Sqrt,
                                 scale=float(XQS * XQS))
            nc.vector.reciprocal(out=minv[:], in_=sqn[:])

            for j in range(JN):
                xntc = xntp.tile([P, KD, CN], FP8, tag="xntc")
                aAc = [psum.tile([1, CD], FP32, tag="mmps",
                                 name=f"aAc{j}_{c2}") for c2 in range(JD)]
                xc = p1.tile([P, TPC, D], BF16, tag="xc")
                nc.sync.dma_start(
                    out=xc[:],
                    in_=x_h[j * CN:(j + 1) * CN, :].rearrange(
                        "(t p) d -> p t d", p=P))
                for t in range(TPC):
                    i = j * TPC + t
                    xf = xc[:, t, :]
                    # both LN stats on ACT via accum_out: Copy -> mean,
                    # Square -> sum(x^2); DVE stays off the stats path
                    mv = p1s.tile([P, 2], FP32, tag="mv")
                    xcp = p1.tile([P, D], BF16, tag="xcp", bufs=2)
                    nc.scalar.activation(out=xcp[:], in_=xf[:], func=AF.Copy,
                                         scale=1.0 / float(D),
                                         accum_out=mv[:, 0:1])
                    xsq = p1.tile([P, D], BF16, tag="xsq", bufs=2)
                    nc.scalar.activation(out=xsq[:], in_=xf[:], func=AF.Square,
                                         accum_out=ssq_t[:, i:i + 1])
                    m2 = p1s.tile([P, 1], FP32, tag="m2")
                    nc.vector.tensor_mul(m2[:], mv[:, 0:1], mv[:, 0:1])
                    nc.vector.tensor_scalar(out=mv[:, 1:2],
                                            in0=ssq_t[:, i:i + 1],
                                            scalar1=1.0 / float(D),
                                            scalar2=m2[:],
                                            op0=ALU.mult, op1=ALU.subtract)
                    xnb = p1n.tile([P, D], BF16, tag="xnb", bufs=3)
                    if not apply_gamma_beta:
                        den = p1s.tile([P, 1], FP32, tag="den")
                        nc.vector.tensor_scalar_add(den[:], mv[:, 1:2], LN_EPS)
                        rden = p1s.tile([P, 1], FP32, tag="rden")
                        nc.vector.reciprocal(out=rden[:], in_=den[:])
                        w_ = p1s.tile([P, 1], FP32, tag="w_")
                        nc.vector.tensor_mul(w_[:], mv[:, 1:2], rden[:])
                        sq1 = p1s.tile([P, 1], FP32, tag="sq1")
                        nc.scalar.activation(out=sq1[:], in_=mv[:, 1:2],
                                             func=AF.Sqrt, scale=float(D))
                        rc = p1s.tile([P, 1], FP32, tag="rc")
                        nc.vector.reciprocal(out=rc[:], in_=sq1[:])
                        c_ = p1s.tile([P, 1], FP32, tag="c_")
                        nc.vector.tensor_scalar_mul(c_[:], rc[:], s_bc[:])
                        sq2 = p1s.tile([P, 1], FP32, tag="sq2")
                        nc.scalar.activation(out=sq2[:], in_=w_[:], func=AF.Sqrt,
                                             scale=float(D))
                        nc.vector.tensor_scalar_mul(tinv[:, i:i + 1], sq2[:],
                                                    sinv_bc[:])
                        nc.vector.tensor_scalar(out=xnb[:], in0=xf[:],
                                                scalar1=mv[:, 0:1], scalar2=c_[:],
                                                op0=ALU.subtract, op1=ALU.mult)
                    else:
                        lv = p1s.tile([P, 1], FP32, tag="lv")
                        nc.vector.tensor_scalar_add(lv[:], mv[:, 1:2], LN_EPS)
                        q_ = p1s.tile([P, 1], FP32, tag="q_")
                        nc.scalar.activation(out=q_[:], in_=lv[:], func=AF.Sqrt)
                        r = p1s.tile([P, 1], FP32, tag="r")
                        nc.vector.reciprocal(out=r[:], in_=q_[:])
                        xln = p1.tile([P, D], FP32, tag="xln")
                        nc.vector.tensor_scalar(out=xln[:], in0=xf[:],
                                                scalar1=mv[:, 0:1], scalar2=r[:],
                                                op0=ALU.subtract, op1=ALU.mult)
                        nc.vector.tensor_mul(xln[:], xln[:], gm_bc[:])
                        nc.vector.tensor_add(xln[:], xln[:], bt_bc[:])
                        sq = p1.tile([P, D], FP32, tag="sq")
                        nc.vector.tensor_mul(sq[:], xln[:], xln[:])
                        ss = p1s.tile([P, 1], FP32, tag="ss")
                        nc.vector.tensor_reduce(out=ss[:], in_=sq[:], axis=AX.X,
                                                op=ALU.add)
                        qs = p1s.tile([P, 1], FP32, tag="qs")
                        nc.scalar.activation(out=qs[:], in_=ss[:], func=AF.Sqrt)
                        u_ = p1s.tile([P, 1], FP32, tag="u_")
                        nc.vector.reciprocal(out=u_[:], in_=qs[:])
                        t_ = p1s.tile([P, 1], FP32, tag="t_")
                        nc.vector.tensor_scalar_mul(t_[:], u_[:], s_bc[:])
                        nc.vector.reciprocal(out=tinv[:, i:i + 1], in_=t_[:])
                        nc.vector.tensor_scalar_mul(xnb[:], xln[:], t_[:])
                    nc.vector.tensor_scalar_mul(xq[:, i, :], xnb[:], XQS)
                    tvb = p1s.tile([P, 1], BF16, tag="tvb")
                    nc.vector.tensor_scalar_mul(tvb[:], tinv[:, i:i + 1], 1.0)
                    for c2 in range(JD):
                        nc.tensor.matmul(aAc[c2][:], tvb[:],
                                         xnb[:, c2 * CD:(c2 + 1) * CD],
                                         start=(t == 0), stop=(t == TPC - 1),
                                         skip_group_check=True)
                    for k in range(KD):
                        pst = psum.tile([P, P], BF16, tag="pst",
                                        name=f"pxt{i}_{k}", bufs=2)
                        nc.tensor.transpose(pst[:], xnb[:, k * P:(k + 1) * P],
                                            ident_b[:])
                        nc.vector.tensor_scalar_mul(
                            xntc[:, k, t * P:(t + 1) * P], pst[:], XQS)
                for c2 in range(JD):
                    nc.vector.tensor_add(Aacc[:, c2 * CD:(c2 + 1) * CD],
                                         Aacc[:, c2 * CD:(c2 + 1) * CD],
                                         aAc[c2][:])
                # logits (fp8 DoubleRow) + exp + sc rowsum accumulation
                scps = psum.tile([1, CN], FP32, tag="mmps", name=f"scps{j}")
                for e in range(NE):
                    ps = psum.tile([P, CN], FP32, tag="mmps",
                                   name=f"lgps{e}_{j}")
                    for r in range(0, KD, 2):
                        b, rr = r // MUB, r % MUB
                        nc.tensor.matmul(ps[:],
                                         mubc[b][:, rr:rr + 2,
                                                 e * P:(e + 1) * P],
                                         xntc[:, r:r + 2, :],
                                         start=(r == 0), stop=(r == KD - 2),
                                         perf_mode=DR)
                    ett = p2b.tile([P, CN], BF16, tag="ett")
                    nc.scalar.activation(out=ett[:], in_=ps[:], func=AF.Exp,
                                         scale=minv[:, e:e + 1],
                                         accum_out=sdall[:, e * JN + j:
                                                         e * JN + j + 1])
                    nc.tensor.matmul(scps[:], ones_b[:], ett[:],
                                     start=(e == 0), stop=(e == NE - 1),
                                     skip_group_check=True)
                    nc.sync.dma_start(
                        out=et_d[e * P:(e + 1) * P, j * CN:(j + 1) * CN],
                        in_=ett[:])
                nc.scalar.copy(out=sc_sb[:, j * CN:(j + 1) * CN], in_=scps[:])
            for e in range(NE):
                nc.vector.tensor_reduce(
                    out=sd[:, e:e + 1],
                    in_=sdall[:, e * JN:(e + 1) * JN], axis=AX.X, op=ALU.add)
            nc.vector.reciprocal(out=sdinv[:], in_=sd[:])
            # sc columns: [1,N] row -> [P, NT] via PE transposes; fold the
            # 1/(EDS*SOS) combine scale into the reciprocal.
            nc.vector.tensor_scalar_mul(A_sb[:], Aacc[:], float(XQS))
            for i in range(NT):
                pstc = psum.tile([P, 1], FP32, tag="pst", name=f"pstc{i}",
                                 bufs=2)
                nc.tensor.transpose(pstc[:], sc_sb[:1, i * P:(i + 1) * P],
                                    ident_f[:1, :1])
                nc.vector.tensor_copy(out=scv[:, i:i + 1], in_=pstc[:])
            nc.vector.tensor_scalar_mul(scv[:], scv[:], float(EDS * SOS))
            nc.vector.reciprocal(out=scinvq[:], in_=scv[:])
        mub_ctx.close()

        # ------------- P3: A-row, xq, then pipelined dispatch + MLP ---------
        # fp8 SOS*so in [es-part, es-tile, d] layout: combine DoubleRow rhs
        # (opened before the P3 pools: it outlives them, LIFO release order)
        soqp = ctx.enter_context(tc.tile_pool(name="soq_pool", bufs=1))
        soq = soqp.tile([P, NE, D], FP8, tag="soq")
        p3_ctx = ExitStack()
        sitp = p3_ctx.enter_context(tc.tile_pool(name="sit_pool", bufs=1))
        echqp = p3_ctx.enter_context(tc.tile_pool(name="echq", bufs=2))
        echtp = p3_ctx.enter_context(tc.tile_pool(name="echt", bufs=4))
        mlp = p3_ctx.enter_context(tc.tile_pool(name="mlp", bufs=2))
        mlpw1 = p3_ctx.enter_context(tc.tile_pool(name="mlp_w1", bufs=2))
        mlpw2 = p3_ctx.enter_context(tc.tile_pool(name="mlp_w2", bufs=2))
        mlpsm = p3_ctx.enter_context(tc.tile_pool(name="mlp_sm", bufs=4))
        sobp = p3_ctx.enter_context(tc.tile_pool(name="sob", bufs=2))

        siT2 = [[sitp.tile([P, CE], BF16, tag=f"siT{par}_{d}",
                           name=f"siT{par}_{d}") for d in range(KD)]
                for par in range(2)]
        gelu_f = AF.Tanh if SIM_SAFE_GELU else AF.Gelu

        def mlp_expert(e, par):
            le = e % EPC
            KH = KD // 2
            w1e = [mlpw1.tile([P, KH, H], BF16, tag="w1e", bufs=2,
                              name=f"w1e{e}_{hh}") for hh in range(2)]
            for hh in range(2):
                nc.sync.dma_start(
                    out=w1e[hh][:],
                    in_=w1_h[e, hh * KH * P:(hh + 1) * KH * P, :]
                    .rearrange("(k p) h -> p k h", p=P))
            w2e = [mlpw2.tile([P, QH // 2, D], BF16, tag="w2e", bufs=2,
                              name=f"w2e{e}_{hh}") for hh in range(2)]
            for hh in range(2):
                nc.scalar.dma_start(
                    out=w2e[hh][:],
                    in_=w2_h[e, hh * (QH // 2) * P:(hh + 1) * (QH // 2) * P, :]
                    .rearrange("(q p) d -> p q d", p=P))
            psh = psum.tile([P, H], FP32, tag="mmps", name=f"psh{e}")
            for k in range(KD):
                nc.tensor.matmul(psh[:],
                                 siT2[par][k][:, le * P:(le + 1) * P],
                                 w1e[k // KH][:, k % KH, :],
                                 start=(k == 0),
                                 stop=(k == KD - 1 and not apply_b1))
            if apply_b1:
                pst0 = psum.tile([P, P], FP32, tag="pst", name=f"psdr{e}",
                                 bufs=2)
                nc.tensor.transpose(pst0[:1, :], sd[:, e:e + 1], ident_f[:])
                sdrow = mlpsm.tile([1, P], BF16, tag="sdrow")
                nc.vector.tensor_copy(out=sdrow[:], in_=pst0[:1, :])
                b1row = mlpsm.tile([1, H], BF16, tag="b1row")
                nc.gpsimd.dma_start(out=b1row[:], in_=b1_h[e:e + 1, :])
                nc.tensor.matmul(psh[:], sdrow[:], b1row[:],
                                 start=False, stop=True)
            hbf = mlp.tile([P, H], BF16, tag="hbf", bufs=2)
            nc.scalar.activation(out=hbf[:], in_=psh[:], func=gelu_f,
                                 scale=sdinv[:, e:e + 1])
            hT = mlp.tile([P, QH, P], BF16, tag="hT", bufs=2)
            for q in range(QH):
                pst = psum.tile([P, P], BF16, tag="pst", name=f"pst{e}_{q}",
                                bufs=2)
                nc.tensor.transpose(pst[:], hbf[:, q * P:(q + 1) * P],
                                    ident_b[:])
                nc.vector.tensor_copy(out=hT[:, q, :], in_=pst[:])
            if apply_b2:
                b2row = mlpsm.tile([1, D], BF16, tag="b2row")
                nc.gpsimd.dma_start(out=b2row[:], in_=b2_h[e:e + 1, :])
            soe = sobp.tile([P, D], BF16, tag="sob", bufs=2)
            for dch in range(JD):
                pso = psum.tile([P, CD], FP32, tag="mmps",
                                name=f"pso{e}_{dch}")
                for q in range(QH):
                    nc.tensor.matmul(
                        pso[:], hT[:, q, :],
                        w2e[q // (QH // 2)][:, q % (QH // 2),
                                            dch * CD:(dch + 1) * CD],
                        start=(q == 0), stop=(q == QH - 1 and not apply_b2))
                if apply_b2:
                    nc.tensor.matmul(
                        pso[:], ones_row[:1, :P],
                        b2row[:, dch * CD:(dch + 1) * CD],
                        start=False, stop=True)
                nc.vector.tensor_copy(
                    out=soe[:, dch * CD:(dch + 1) * CD], in_=pso[:])
            # fp8 copy for the combine + colsum(so) accumulation for A2
            nc.vector.tensor_scalar_mul(soq[:, e, :], soe[:], SOS)
            for c2 in range(JD):
                a2t = psum.tile([1, CD], FP32, tag="pst", name=f"a2t{e}_{c2}",
                                bufs=2)
                nc.tensor.matmul(a2t[:], ones_b[:],
                                 soe[:, c2 * CD:(c2 + 1) * CD])
                nc.vector.tensor_add(A2acc[:, c2 * CD:(c2 + 1) * CD],
                                     A2acc[:, c2 * CD:(c2 + 1) * CD], a2t[:])

        for c in range(JE):
            echq = echqp.tile([P, NT, CE], FP8, tag="echq")
            for k in range(NT):
                echt = echtp.tile([P, CE], BF16, tag="echt", bufs=4)
                eng = nc.sync if k % 2 == 0 else nc.scalar
                eng.dma_start(
                    out=echt[:],
                    in_=et_d[c * CE:(c + 1) * CE, k * P:(k + 1) * P],
                    transpose=True)
                # (E - 1) * tinv * EDS, fp8
                nc.vector.tensor_scalar(out=echq[:, k, :], in0=echt[:],
                                        scalar1=1.0,
                                        scalar2=tinv[:, k:k + 1],
                                        op0=ALU.subtract, op1=ALU.mult)
            par = c % 2
            prev = list(range((c - 1) * EPC, c * EPC)) if c > 0 else []
            for d in range(KD):
                ps = psum.tile([P, CE], FP32, tag="mmps", name=f"sips{c}_{d}")
                for r in range(0, NT, 2):
                    nc.tensor.matmul(ps[:],
                                     xq[:, r:r + 2, d * P:(d + 1) * P],
                                     echq[:, r:r + 2, :],
                                     start=(r == 0), stop=False,
                                     perf_mode=DR)
                nc.tensor.matmul(ps[:], A_sb[:1, d * P:(d + 1) * P],
                                 ones_row[:], start=False, stop=True)
                nc.vector.tensor_scalar_mul(siT2[par][d][:], ps[:],
                                            1.0 / float(XQS))
                if d % 4 == 3 and prev:
                    mlp_expert(prev[d // 4], 1 - par)
        for e in range((JE - 1) * EPC, JE * EPC):
            mlp_expert(e, (JE - 1) % 2)
        nc.vector.tensor_scalar_mul(A2_sb[:], A2acc[:], float(EDS * SOS))
        p3_ctx.close()

        # ------------- P4: combine (fp8 DoubleRow + rank-1 A2) --------------
        et_view = et_d[:, :].rearrange("(k p) n -> p k n", p=P)
        with tc.tile_pool(name="p4", bufs=3) as p4, \
                tc.tile_pool(name="p4q", bufs=2) as p4q:
            for i in range(NT):
                etb = p4.tile([P, NE, P], BF16, tag="etb")
                nc.sync.dma_start(out=etb[:],
                                  in_=et_view[:, :, i * P:(i + 1) * P])
                etbq = p4q.tile([P, NE, P], FP8, tag="etbq")
                nc.vector.tensor_scalar(out=etbq[:], in0=etb[:], scalar1=1.0,
                                        scalar2=float(EDS),
                                        op0=ALU.subtract, op1=ALU.mult)
                pso_ = [psum.tile([P, CD], FP32, tag="mmps",
                                  name=f"ops{i}_{j}") for j in range(JD)]
                for ki in range(0, NE, 2):
                    for dch in range(JD):
                        nc.tensor.matmul(
                            pso_[dch][:], etbq[:, ki:ki + 2, :],
                            soq[:, ki:ki + 2, dch * CD:(dch + 1) * CD],
                            start=(ki == 0), stop=False, perf_mode=DR)
                for dch in range(JD):
                    nc.tensor.matmul(pso_[dch][:], ones_row[:1, :P],
                                     A2_sb[:1, dch * CD:(dch + 1) * CD],
                                     start=False, stop=True)
                outt = p4.tile([P, D], FP32, tag="outt")
                for dch in range(JD):
                    nc.scalar.activation(
                        out=outt[:, dch * CD:(dch + 1) * CD],
                        in_=pso_[dch][:], func=AF.Copy,
                        scale=scinvq[:, i:i + 1])
                nc.sync.dma_start(out=out_h[i * P:(i + 1) * P, :],
                                  in_=outt[:])
    nc.compile()
    return nc


_NC_CACHE = {}


def _get_nc(N, D, E, S, H, flags):
    key = (N, D, E, S, H, flags)
    if key not in _NC_CACHE:
        _NC_CACHE[key] = build_softmoe(
            N, D, E, S, H, apply_gamma_beta=flags[0], apply_b1=flags[1],
            apply_b2=flags[2])
    return _NC_CACHE[key]


def kernel(x, gamma, beta, mu, scale, W1, b1, W2, b2):
    import ml_dtypes
    from concourse.bass_utils import run_bass_kernel_spmd

    BF = ml_dtypes.bfloat16
    F8 = ml_dtypes.float8_e4m3
    x = np.asarray(x, dtype=np.float32)
    gamma = np.ascontiguousarray(np.asarray(gamma, dtype=np.float32))
    beta = np.ascontiguousarray(np.asarray(beta, dtype=np.float32))
    mu = np.asarray(mu, dtype=np.float32)
    scale = np.ascontiguousarray(np.asarray(scale, dtype=np.float32))
    W1 = np.asarray(W1, dtype=np.float32)
    b1 = np.ascontiguousarray(np.asarray(b1, dtype=np.float32))
    W2 = np.asarray(W2, dtype=np.float32)
    b2 = np.ascontiguousarray(np.asarray(b2, dtype=np.float32))

    B, N, D = x.shape
    _, E, S = mu.shape
    H = W1.shape[2]
    n_cores = 8
    assert B == n_cores, f"kernel hardcoded for B == {n_cores}, got {B}"

    flags = (
        bool(np.any(gamma != 1.0) or np.any(beta != 0.0)
             or np.any(scale <= 0.0)),
        bool(np.any(b1 != 0.0)),
        bool(np.any(b2 != 0.0)),
    )
    nc = _get_nc(N, D, E, S, H, flags)

    xb = np.ascontiguousarray(x.astype(BF))
    muq = np.ascontiguousarray(
        np.clip(mu * MUS, -240.0, 240.0).astype(F8))
    W1b = np.ascontiguousarray(W1.astype(BF))
    W2b = np.ascontiguousarray(W2.astype(BF))

    shared = dict(gamma=gamma, beta=beta, mu=muq, scale=scale, W1=W1b, b1=b1,
                  W2=W2b, b2=b2)
    in_maps = [dict(x=xb[b], **shared) for b in range(n_cores)]
    import os
    trace = bool(os.environ.get("SOFTMOE_TRACE"))
    res = run_bass_kernel_spmd(nc, in_maps, core_ids=list(range(n_cores)),
                               trace=trace)
    global LAST_RESULT
    LAST_RESULT = res
    return np.stack([r["out"] for r in res.results], axis=0)


LAST_RESULT = None


# revision 31
# speedup vs baseline: 1.1202x; 1.1202x over previous
"""Trainium2 Bass kernel for SoftMoE (LayerNorm + cosine routing + per-expert MLP).

Sharding: pure data-parallel over batch B=8 -> one batch element per NeuronCore.
No collectives. Each core computes its full (N, D) output slice.

Math notes (per core, x is (N, D)):
  x_ln = LN(x) * gamma + beta
  x_n  = x_ln * t[n],   t[n] = scale / ||x_ln[n]||
  logitsT[es, n] = minv[es] * (mu_q.T @ xq.T)  (mu_q = 32*mu fp8, xq = 32*x_n
                   fp8, minv = 1/(1024*||mu||) folds both 32x scales)
  E = exp(logits); dispatch = E/sd[es] (softmax over n); combine = E/sc[n]
  slot_raw[d,es] = sum_n x_ln[n,d] E[n,es] = A[d] + sum_n x_ln (E-1)
  h  = gelu(sdinv[es] * slot_raw @ W1 + b1);  so = h @ W2 + b2
  out[n] = (A2[d] + sum_es (E-1) so[es,d]) / sc[n],  A2 = colsum(so)
The (E-1)-centered forms let dispatch/combine run as fp8 DoubleRow matmuls
(E~1 would vanish under fp8 quantization); the rank-1 A/A2 corrections are
cheap K=1 bf16 matmuls. sd comes from the exp eviction's accum_out; sc from
ones-lhsT rowsum matmuls over E during the logits phase.

Staging: x/W1/W2 bf16, mu fp8e4 (x32 pre-scale) -- host-side casts. MLP stays
bf16 (weight quantization error would be correlated across slots and not
average out in the combine). The per-expert MLPs are software-pipelined into
the NEXT dispatch chunk's matmul stream so the 4MB/expert weight DMA is
consumed evenly instead of in bursts.
"""

import numpy as np
from contextlib import ExitStack

import concourse.bass as bass
import concourse.tile as tile
from concourse import bacc
from concourse import mybir
from concourse.masks import make_identity

FP32 = mybir.dt.float32
BF16 = mybir.dt.bfloat16
FP8 = mybir.dt.float8e4
DR = mybir.MatmulPerfMode.DoubleRow
AF = mybir.ActivationFunctionType
ALU = mybir.AluOpType
AX = mybir.AxisListType

P = 128
LN_EPS = 1e-5
MUS = 32.0    # host-side mu pre-scale (fp8 range centering)
XQS = 32.0    # on-device x_n pre-scale for fp8 logits / dispatch lhsT
EDS = 64.0    # (E-1) pre-scale for fp8 dispatch/combine moving operands
SOS = 16.0    # slot_out pre-scale for the fp8 combine rhs

SIM_SAFE_GELU = False


def _bcast_ap(handle, p, free):
    """AP reading a 1-D DRAM tensor broadcast across p partitions."""
    return bass.AP(tensor=handle, offset=0, ap=[[0, p], [1, free]])


def build_softmoe(N, D, E, S, H, *, apply_gamma_beta=True, apply_b1=True,
                  apply_b2=True):
    assert S == P
    ES = E * S
    NT, KD, NE, QH = N // P, D // P, ES // P, H // P
    CN = min(512, N); JN = N // CN       # n-chunks
    CE = min(512, ES); JE = ES // CE     # es-chunks
    CD = min(512, D); JD = D // CD       # d-chunks
    EPC = CE // P                        # experts per es-chunk
    TPC = CN // P                        # P-tiles per n-chunk
    MUB = 4                              # mu k-tiles per load batch

    nc = bacc.Bacc(None, target_bir_lowering=False, debug=False)

    x_h = nc.dram_tensor("x", [N, D], BF16, kind="ExternalInput")
    g_h = nc.dram_tensor("gamma", [D], FP32, kind="ExternalInput")
    be_h = nc.dram_tensor("beta", [D], FP32, kind="ExternalInput")
    mu_h = nc.dram_tensor("mu", [D, E, S], FP8, kind="ExternalInput")
    sc_h = nc.dram_tensor("scale", [1], FP32, kind="ExternalInput")
    w1_h = nc.dram_tensor("W1", [E, D, H], BF16, kind="ExternalInput")
    b1_h = nc.dram_tensor("b1", [E, H], FP32, kind="ExternalInput")
    w2_h = nc.dram_tensor("W2", [E, H, D], BF16, kind="ExternalInput")
    b2_h = nc.dram_tensor("b2", [E, D], FP32, kind="ExternalInput")
    out_h = nc.dram_tensor("out", [N, D], FP32, kind="ExternalOutput")

    et_d = nc.dram_tensor("et_scr", [ES, N], BF16)

    with tile.TileContext(nc, pool_alloc_mode="queue") as tc, ExitStack() as ctx:
        small = ctx.enter_context(tc.tile_pool(name="small", bufs=1))
        psum = ctx.enter_context(tc.tile_pool(name="psum", bufs=6, space="PSUM"))

        ones_b = small.tile([P, 1], BF16, tag="ones_b")
        nc.vector.memset(ones_b, 1.0)
        ones_row = small.tile([1, CE], BF16, tag="ones_row")
        nc.vector.memset(ones_row, 1.0)
        s_bc = small.tile([P, 1], FP32, tag="s_bc")
        nc.gpsimd.dma_start(out=s_bc, in_=_bcast_ap(sc_h, P, 1))
        tinv = small.tile([P, NT], FP32, tag="tinv")
        minv = small.tile([P, NE], FP32, tag="minv")
        sd = small.tile([P, NE], FP32, tag="sd")
        sdinv = small.tile([P, NE], FP32, tag="sdinv")
        scv = small.tile([P, NT], FP32, tag="scv")
        scinvq = small.tile([P, NT], FP32, tag="scinvq")
        ident_b = small.tile([P, P], BF16, tag="ident_b")
        make_identity(nc, ident_b)
        ident_f = small.tile([P, P], FP32, tag="ident_f")
        make_identity(nc, ident_f)
        if apply_gamma_beta:
            gm_bc = small.tile([P, D], FP32, tag="gm_bc")
            nc.gpsimd.dma_start(out=gm_bc, in_=_bcast_ap(g_h, P, D))
            bt_bc = small.tile([P, D], FP32, tag="bt_bc")
            nc.gpsimd.dma_start(out=bt_bc, in_=_bcast_ap(be_h, P, D))

        rows = ctx.enter_context(tc.tile_pool(name="rows", bufs=1))
        n2c = small.tile([P, NE], FP32, tag="n2c")
        A_sb = rows.tile([1, D], BF16, tag="A_sb")
        Aacc = rows.tile([1, D], FP32, tag="Aacc")
        nc.vector.memset(Aacc, 0.0)
        A2acc = rows.tile([1, D], FP32, tag="A2acc")
        nc.vector.memset(A2acc, 0.0)
        A2_sb = rows.tile([1, D], BF16, tag="A2_sb")
        # fp8 32*x_n in [n-part, n-tile, d] layout: resident through P3 as
        # the dispatch DoubleRow lhsT and the transpose source for logits
        xqp = ctx.enter_context(tc.tile_pool(name="xq_pool", bufs=1))
        xq = xqp.tile([P, NT, D], FP8, tag="xq")
        mub_ctx = ExitStack()
        mubp = mub_ctx.enter_context(tc.tile_pool(name="mub_pool", bufs=1))
        mubc = [mubp.tile([P, MUB, ES], FP8, tag=f"mubc{b}", name=f"mubc{b}")
                for b in range(KD // MUB)]

        # ------------- fused P1+P2: LN -> x_nT (PE transpose) -> logits ------
        sinv_bc = small.tile([P, 1], FP32, tag="sinv_bc")
        nc.vector.reciprocal(out=sinv_bc[:], in_=s_bc[:])
        sdall = small.tile([P, NE * JN], FP32, tag="sdall")
        ssq_t = small.tile([P, NT], FP32, tag="ssq_t")
        with tc.tile_pool(name="p1", bufs=2) as p1, \
                tc.tile_pool(name="p1n", bufs=3) as p1n, \
                tc.tile_pool(name="p1s", bufs=8) as p1s, \
                tc.tile_pool(name="rows_f", bufs=1) as rows_f, \
                tc.tile_pool(name="p2a", bufs=2) as p2a, \
                tc.tile_pool(name="xnT_pool", bufs=3) as xntp, \
                tc.tile_pool(name="p2b", bufs=6) as p2b:
            arow_sb = rows_f.tile([1, ES], FP32, tag="arow_sb")
            sc_sb = rows_f.tile([1, N], FP32, tag="sc_sb")
            for b in range(KD // MUB):
                nc.scalar.dma_start(
                    out=mubc[b][:],
                    in_=mu_h[b * MUB * P:(b + 1) * MUB * P].rearrange(
                        "(k p) e s -> p k (e s)", p=P))
            # mu column sq-norms as rows (cheap 1-col-LDW rowsum matmuls),
            # then 16 tiny PE transposes back to column layout.
            arow = [psum.tile([1, CE], FP32, tag="mmps", name=f"arow{c}")
                    for c in range(JE)]
            for k in range(KD):
                msq = p2a.tile([P, ES], BF16, tag="msq")
                nc.vector.tensor_mul(msq[:], mubc[k // MUB][:, k % MUB, :],
                                     mubc[k // MUB][:, k % MUB, :])
                for c in range(JE):
                    nc.tensor.matmul(arow[c][:], ones_b[:],
                                     msq[:, c * CE:(c + 1) * CE],
                                     start=(k == 0), stop=(k == KD - 1),
                                     skip_group_check=True)
            for c in range(JE):
                nc.scalar.copy(out=arow_sb[:, c * CE:(c + 1) * CE],
                               in_=arow[c][:])
            for e in range(NE):
                pstn = psum.tile([P, 1], FP32, tag="pst", name=f"pstn{e}",
                                 bufs=2)
                nc.tensor.transpose(pstn[:], arow_sb[:1, e * P:(e + 1) * P],
                                    ident_f[:1, :1])
                nc.vector.tensor_copy(out=n2c[:, e:e + 1], in_=pstn[:])
            sqn = small.tile([P, NE], FP32, tag="sqn")
            nc.scalar.activation(out=sqn[:], in_=n2c[:], func=AF.Sqrt,
                                 scale=float(XQS * XQS))
            nc.vector.reciprocal(out=minv[:], in_=sqn[:])

            for j in range(JN):
                xntc = xntp.tile([P, KD, CN], FP8, tag="xntc")
                aAc = [psum.tile([1, CD], FP32, tag="mmps",
                                 name=f"aAc{j}_{c2}") for c2 in range(JD)]
                xc = p1.tile([P, TPC, D], BF16, tag="xc")
                nc.sync.dma_start(
                    out=xc[:],
                    in_=x_h[j * CN:(j + 1) * CN, :].rearrange(
                        "(t p) d -> p t d", p=P))
                for t in range(TPC):
                    i = j * TPC + t
                    xf = xc[:, t, :]
                    # both LN stats on ACT via accum_out: Copy -> mean,
                    # Square -> sum(x^2); DVE stays off the stats path
                    mv = p1s.tile([P, 2], FP32, tag="mv")
                    xcp = p1.tile([P, D], BF16, tag="xcp", bufs=2)
                    nc.scalar.activation(out=xcp[:], in_=xf[:], func=AF.Copy,
                                         scale=1.0 / float(D),
                                         accum_out=mv[:, 0:1])
                    xsq = p1.tile([P, D], BF16, tag="xsq", bufs=2)
                    nc.scalar.activation(out=xsq[:], in_=xf[:], func=AF.Square,
                                         accum_out=ssq_t[:, i:i + 1])
                    m2 = p1s.tile([P, 1], FP32, tag="m2")
                    nc.vector.tensor_mul(m2[:], mv[:, 0:1], mv[:, 0:1])
                    nc.vector.tensor_scalar(out=mv[:, 1:2],
                                            in0=ssq_t[:, i:i + 1],
                                            scalar1=1.0 / float(D),
                                            scalar2=m2[:],
                                            op0=ALU.mult, op1=ALU.subtract)
                    xnb = p1n.tile([P, D], BF16, tag="xnb", bufs=3)
                    if not apply_gamma_beta:
                        den = p1s.tile([P, 1], FP32, tag="den")
                        nc.vector.tensor_scalar_add(den[:], mv[:, 1:2], LN_EPS)
                        rden = p1s.tile([P, 1], FP32, tag="rden")
                        nc.vector.reciprocal(out=rden[:], in_=den[:])
                        w_ = p1s.tile([P, 1], FP32, tag="w_")
                        nc.vector.tensor_mul(w_[:], mv[:, 1:2], rden[:])
                        sq1 = p1s.tile([P, 1], FP32, tag="sq1")
                        nc.scalar.activation(out=sq1[:], in_=mv[:, 1:2],
                                             func=AF.Sqrt, scale=float(D))
                        rc = p1s.tile([P, 1], FP32, tag="rc")
                        nc.vector.reciprocal(out=rc[:], in_=sq1[:])
                        c_ = p1s.tile([P, 1], FP32, tag="c_")
                        nc.vector.tensor_scalar_mul(c_[:], rc[:], s_bc[:])
                        sq2 = p1s.tile([P, 1], FP32, tag="sq2")
                        nc.scalar.activation(out=sq2[:], in_=w_[:], func=AF.Sqrt,
                                             scale=float(D))
                        nc.vector.tensor_scalar_mul(tinv[:, i:i + 1], sq2[:],
                                                    sinv_bc[:])
                        nc.vector.tensor_scalar(out=xnb[:], in0=xf[:],
                                                scalar1=mv[:, 0:1], scalar2=c_[:],
                                                op0=ALU.subtract, op1=ALU.mult)
                    else:
                        lv = p1s.tile([P, 1], FP32, tag="lv")
                        nc.vector.tensor_scalar_add(lv[:], mv[:, 1:2], LN_EPS)
                        q_ = p1s.tile([P, 1], FP32, tag="q_")
                        nc.scalar.activation(out=q_[:], in_=lv[:], func=AF.Sqrt)
                        r = p1s.tile([P, 1], FP32, tag="r")
                        nc.vector.reciprocal(out=r[:], in_=q_[:])
                        xln = p1.tile([P, D], FP32, tag="xln")
                        nc.vector.tensor_scalar(out=xln[:], in0=xf[:],
                                                scalar1=mv[:, 0:1], scalar2=r[:],
                                                op0=ALU.subtract, op1=ALU.mult)
                        nc.vector.tensor_mul(xln[:], xln[:], gm_bc[:])
                        nc.vector.tensor_add(xln[:], xln[:], bt_bc[:])
                        sq = p1.tile([P, D], FP32, tag="sq")
                        nc.vector.tensor_mul(sq[:], xln[:], xln[:])
                        ss = p1s.tile([P, 1], FP32, tag="ss")
                        nc.vector.tensor_reduce(out=ss[:], in_=sq[:], axis=AX.X,
                                                op=ALU.add)
                        qs = p1s.tile([P, 1], FP32, tag="qs")
                        nc.scalar.activation(out=qs[:], in_=ss[:], func=AF.Sqrt)
                        u_ = p1s.tile([P, 1], FP32, tag="u_")
                        nc.vector.reciprocal(out=u_[:], in_=qs[:])
                        t_ = p1s.tile([P, 1], FP32, tag="t_")
                        nc.vector.tensor_scalar_mul(t_[:], u_[:], s_bc[:])
                        nc.vector.reciprocal(out=tinv[:, i:i + 1], in_=t_[:])
                        nc.vector.tensor_scalar_mul(xnb[:], xln[:], t_[:])
                    nc.vector.tensor_scalar_mul(xq[:, i, :], xnb[:], XQS)
                    tvb = p1s.tile([P, 1], BF16, tag="tvb")
                    nc.vector.tensor_scalar_mul(tvb[:], tinv[:, i:i + 1], 1.0)
                    for c2 in range(JD):
                        nc.tensor.matmul(aAc[c2][:], tvb[:],
                                         xnb[:, c2 * CD:(c2 + 1) * CD],
                                         start=(t == 0), stop=(t == TPC - 1),
                                         skip_group_check=True)
                    for k in range(KD):
                        pst = psum.tile([P, P], BF16, tag="pst",
                                        name=f"pxt{i}_{k}", bufs=2)
                        nc.tensor.transpose(pst[:], xnb[:, k * P:(k + 1) * P],
                                            ident_b[:])
                        nc.vector.tensor_scalar_mul(
                            xntc[:, k, t * P:(t + 1) * P], pst[:], XQS)
                for c2 in range(JD):
                    nc.vector.tensor_add(Aacc[:, c2 * CD:(c2 + 1) * CD],
                                         Aacc[:, c2 * CD:(c2 + 1) * CD],
                                         aAc[c2][:])
                # logits (fp8 DoubleRow) + exp + sc rowsum accumulation
                scps = psum.tile([1, CN], FP32, tag="mmps", name=f"scps{j}")
                for e in range(NE):
                    ps = psum.tile([P, CN], FP32, tag="mmps",
                                   name=f"lgps{e}_{j}")
                    for r in range(0, KD, 2):
                        b, rr = r // MUB, r % MUB
                        nc.tensor.matmul(ps[:],
                                         mubc[b][:, rr:rr + 2,
                                                 e * P:(e + 1) * P],
                                         xntc[:, r:r + 2, :],
                                         start=(r == 0), stop=(r == KD - 2),
                                         perf_mode=DR)
                    ett = p2b.tile([P, CN], BF16, tag="ett")
                    nc.scalar.activation(out=ett[:], in_=ps[:], func=AF.Exp,
                                         scale=minv[:, e:e + 1],
                                         accum_out=sdall[:, e * JN + j:
                                                         e * JN + j + 1])
                    nc.tensor.matmul(scps[:], ones_b[:], ett[:],
                                     start=(e == 0), stop=(e == NE - 1),
                                     skip_group_check=True)
                    nc.sync.dma_start(
                        out=et_d[e * P:(e + 1) * P, j * CN:(j + 1) * CN],
                        in_=ett[:])
                nc.scalar.copy(out=sc_sb[:, j * CN:(j + 1) * CN], in_=scps[:])
            for e in range(NE):
                nc.vector.tensor_reduce(
                    out=sd[:, e:e + 1],
                    in_=sdall[:, e * JN:(e + 1) * JN], axis=AX.X, op=ALU.add)
            nc.vector.reciprocal(out=sdinv[:], in_=sd[:])
            # sc columns: [1,N] row -> [P, NT] via PE transposes; fold the
            # 1/(EDS*SOS) combine scale into the reciprocal.
            nc.vector.tensor_scalar_mul(A_sb[:], Aacc[:], float(XQS))
            for i in range(NT):
                pstc = psum.tile([P, 1], FP32, tag="pst", name=f"pstc{i}",
                                 bufs=2)
                nc.tensor.transpose(pstc[:], sc_sb[:1, i * P:(i + 1) * P],
                                    ident_f[:1, :1])
                nc.vector.tensor_copy(out=scv[:, i:i + 1], in_=pstc[:])
            nc.vector.tensor_scalar_mul(scv[:], scv[:], float(EDS * SOS))
            nc.vector.reciprocal(out=scinvq[:], in_=scv[:])
        mub_ctx.close()

        # ------------- P3: A-row, xq, then pipelined dispatch + MLP ---------
        # fp8 SOS*so in [es-part, es-tile, d] layout: combine DoubleRow rhs
        # (opened before the P3 pools: it outlives them, LIFO release order)
        soqp = ctx.enter_context(tc.tile_pool(name="soq_pool", bufs=1))
        soq = soqp.tile([P, NE, D], FP8, tag="soq")
        p3_ctx = ExitStack()
        sitp = p3_ctx.enter_context(tc.tile_pool(name="sit_pool", bufs=1))
        echqp = p3_ctx.enter_context(tc.tile_pool(name="echq", bufs=2))
        echtp = p3_ctx.enter_context(tc.tile_pool(name="echt", bufs=4))
        mlp = p3_ctx.enter_context(tc.tile_pool(name="mlp", bufs=2))
        mlpw1 = p3_ctx.enter_context(tc.tile_pool(name="mlp_w1", bufs=2))
        mlpw2 = p3_ctx.enter_context(tc.tile_pool(name="mlp_w2", bufs=2))
        mlpsm = p3_ctx.enter_context(tc.tile_pool(name="mlp_sm", bufs=4))
        sobp = p3_ctx.enter_context(tc.tile_pool(name="sob", bufs=2))

        siT2 = [[sitp.tile([P, CE], BF16, tag=f"siT{par}_{d}",
                           name=f"siT{par}_{d}") for d in range(KD)]
                for par in range(2)]
        gelu_f = AF.Tanh if SIM_SAFE_GELU else AF.Gelu

        def mlp_expert(e, par):
            le = e % EPC
            KH = KD // 2
            w1e = [mlpw1.tile([P, KH, H], BF16, tag="w1e", bufs=2,
                              name=f"w1e{e}_{hh}") for hh in range(2)]
            for hh in range(2):
                nc.sync.dma_start(
                    out=w1e[hh][:],
                    in_=w1_h[e, hh * KH * P:(hh + 1) * KH * P, :]
                    .rearrange("(k p) h -> p k h", p=P))
            w2e = [mlpw2.tile([P, QH // 2, D], BF16, tag="w2e", bufs=2,
                              name=f"w2e{e}_{hh}") for hh in range(2)]
            for hh in range(2):
                nc.scalar.dma_start(
                    out=w2e[hh][:],
                    in_=w2_h[e, hh * (QH // 2) * P:(hh + 1) * (QH // 2) * P, :]
                    .rearrange("(q p) d -> p q d", p=P))
            psh = psum.tile([P, H], FP32, tag="mmps", name=f"psh{e}")
            for k in range(KD):
                nc.tensor.matmul(psh[:],
                                 siT2[par][k][:, le * P:(le + 1) * P],
                                 w1e[k // KH][:, k % KH, :],
                                 start=(k == 0),
                                 stop=(k == KD - 1 and not apply_b1))
            if apply_b1:
                pst0 = psum.tile([P, P], FP32, tag="pst", name=f"psdr{e}",
                                 bufs=2)
                nc.tensor.transpose(pst0[:1, :], sd[:, e:e + 1], ident_f[:])
                sdrow = mlpsm.tile([1, P], BF16, tag="sdrow")
                nc.vector.tensor_copy(out=sdrow[:], in_=pst0[:1, :])
                b1row = mlpsm.tile([1, H], BF16, tag="b1row")
                nc.gpsimd.dma_start(out=b1row[:], in_=b1_h[e:e + 1, :])
                nc.tensor.matmul(psh[:], sdrow[:], b1row[:],
                                 start=False, stop=True)
            hbf = mlp.tile([P, H], BF16, tag="hbf", bufs=2)
            nc.scalar.activation(out=hbf[:], in_=psh[:], func=gelu_f,
                                 scale=sdinv[:, e:e + 1])
            hT = mlp.tile([P, QH, P], BF16, tag="hT", bufs=2)
            for q in range(QH):
                pst = psum.tile([P, P], BF16, tag="pst", name=f"pst{e}_{q}",
                                bufs=2)
                nc.tensor.transpose(pst[:], hbf[:, q * P:(q + 1) * P],
                                    ident_b[:])
                nc.vector.tensor_copy(out=hT[:, q, :], in_=pst[:])
            if apply_b2:
                b2row = mlpsm.tile([1, D], BF16, tag="b2row")
                nc.gpsimd.dma_start(out=b2row[:], in_=b2_h[e:e + 1, :])
            soe = sobp.tile([P, D], BF16, tag="sob", bufs=2)
            for dch in range(JD):
                pso = psum.tile([P, CD], FP32, tag="mmps",
                                name=f"pso{e}_{dch}")
                for q in range(QH):
                    nc.tensor.matmul(
                        pso[:], hT[:, q, :],
                        w2e[q // (QH // 2)][:, q % (QH // 2),
                                            dch * CD:(dch + 1) * CD],
                        start=(q == 0), stop=(q == QH - 1 and not apply_b2))
                if apply_b2:
                    nc.tensor.matmul(
                        pso[:], ones_row[:1, :P],
                        b2row[:, dch * CD:(dch + 1) * CD],
                        start=False, stop=True)
                nc.vector.tensor_copy(
                    out=soe[:, dch * CD:(dch + 1) * CD], in_=pso[:])
            # fp8 copy for the combine + colsum(so) accumulation for A2
            nc.vector.tensor_scalar_mul(soq[:, e, :], soe[:], SOS)
            for c2 in range(JD):
                a2t = psum.tile([1, CD], FP32, tag="pst", name=f"a2t{e}_{c2}",
                                bufs=2)
                nc.tensor.matmul(a2t[:], ones_b[:],
                                 soe[:, c2 * CD:(c2 + 1) * CD])
                nc.vector.tensor_add(A2acc[:, c2 * CD:(c2 + 1) * CD],
                                     A2acc[:, c2 * CD:(c2 + 1) * CD], a2t[:])

        for c in range(JE):
            echq = echqp.tile([P, NT, CE], FP8, tag="echq")
            for k in range(NT):
                echt = echtp.tile([P, CE], BF16, tag="echt", bufs=4)
                eng = nc.sync if k % 2 == 0 else nc.scalar
                eng.dma_start(
                    out=echt[:],
                    in_=et_d[c * CE:(c + 1) * CE, k * P:(k + 1) * P],
                    transpose=True)
                # (E - 1) * tinv * EDS, fp8
                nc.vector.tensor_scalar(out=echq[:, k, :], in0=echt[:],
                                        scalar1=1.0,
                                        scalar2=tinv[:, k:k + 1],
                                        op0=ALU.subtract, op1=ALU.mult)
            par = c % 2
            prev = list(range((c - 1) * EPC, c * EPC)) if c > 0 else []
            for d in range(KD):
                ps = psum.tile([P, CE], FP32, tag="mmps", name=f"sips{c}_{d}")
                for r in range(0, NT, 2):
                    nc.tensor.matmul(ps[:],
                                     xq[:, r:r + 2, d * P:(d + 1) * P],
                                     echq[:, r:r + 2, :],
                                     start=(r == 0), stop=False,
                                     perf_mode=DR)
                nc.tensor.matmul(ps[:], A_sb[:1, d * P:(d + 1) * P],
                                 ones_row[:], start=False, stop=True)
                nc.vector.tensor_scalar_mul(siT2[par][d][:], ps[:],
                                            1.0 / float(XQS))
                if d % 4 == 3 and prev:
                    mlp_expert(prev[d // 4], 1 - par)
        for e in range((JE - 1) * EPC, JE * EPC):
            mlp_expert(e, (JE - 1) % 2)
        nc.vector.tensor_scalar_mul(A2_sb[:], A2acc[:], float(EDS * SOS))
        p3_ctx.close()

        # ------------- P4: combine (fp8 DoubleRow + rank-1 A2) --------------
        et_view = et_d[:, :].rearrange("(k p) n -> p k n", p=P)
        with tc.tile_pool(name="p4", bufs=3) as p4, \
                tc.tile_pool(name="p4q", bufs=2) as p4q:
            for i in range(NT):
                etb = p4.tile([P, NE, P], BF16, tag="etb")
                nc.sync.dma_start(out=etb[:],
                                  in_=et_view[:, :, i * P:(i + 1) * P])
                etbq = p4q.tile([P, NE, P], FP8, tag="etbq")
                nc.vector.tensor_scalar(out=etbq[:], in0=etb[:], scalar1=1.0,
                                        scalar2=float(EDS),
                                        op0=ALU.subtract, op1=ALU.mult)
                pso_ = [psum.tile([P, CD], FP32, tag="mmps",
                                  name=f"ops{i}_{j}") for j in range(JD)]
                for ki in range(0, NE, 2):
                    for dch in range(JD):
                        nc.tensor.matmul(
                            pso_[dch][:], etbq[:, ki:ki + 2, :],
                            soq[:, ki:ki + 2, dch * CD:(dch + 1) * CD],
                            start=(ki == 0), stop=False, perf_mode=DR)
                for dch in range(JD):
                    nc.tensor.matmul(pso_[dch][:], ones_row[:1, :P],
                                     A2_sb[:1, dch * CD:(dch + 1) * CD],
                                     start=False, stop=True)
                outt = p4.tile([P, D], FP32, tag="outt")
                for dch in range(JD):
                    nc.scalar.activation(
                        out=outt[:, dch * CD:(dch + 1) * CD],
                        in_=pso_[dch][:], func=AF.Copy,
                        scale=scinvq[:, i:i + 1])
                nc.sync.dma_start(out=out_h[i * P:(i + 1) * P, :],
                                  in_=outt[:])
    nc.compile()
    return nc


_NC_CACHE = {}


def _get_nc(N, D, E, S, H, flags):
    key = (N, D, E, S, H, flags)
    if key not in _NC_CACHE:
        _NC_CACHE[key] = build_softmoe(
            N, D, E, S, H, apply_gamma_beta=flags[0], apply_b1=flags[1],
            apply_b2=flags[2])
    return _NC_CACHE[key]


def kernel(x, gamma, beta, mu, scale, W1, b1, W2, b2):
    import ml_dtypes
    from concourse.bass_utils import run_bass_kernel_spmd

    BF = ml_dtypes.bfloat16
    F8 = ml_dtypes.float8_e4m3
    x = np.asarray(x, dtype=np.float32)
    gamma = np.ascontiguousarray(np.asarray(gamma, dtype=np.float32))
    beta = np.ascontiguousarray(np.asarray(beta, dtype=np.float32))
    mu = np.asarray(mu, dtype=np.float32)
    scale = np.ascontiguousarray(np.asarray(scale, dtype=np.float32))
    W1 = np.asarray(W1, dtype=np.float32)
    b1 = np.ascontiguousarray(np.asarray(b1, dtype=np.float32))
    W2 = np.asarray(W2, dtype=np.float32)
    b2 = np.ascontiguousarray(np.asarray(b2, dtype=np.float32))

    B, N, D = x.shape
    _, E, S = mu.shape
    H = W1.shape[2]
    n_cores = 8
    assert B == n_cores, f"kernel hardcoded for B == {n_cores}, got {B}"

    flags = (
        bool(np.any(gamma != 1.0) or np.any(beta != 0.0)
             or np.any(scale <= 0.0)),
        bool(np.any(b1 != 0.0)),
        bool(np.any(b2 != 0.0)),
    )
    nc = _get_nc(N, D, E, S, H, flags)

    xb = np.ascontiguousarray(x.astype(BF))
    muq = np.ascontiguousarray(
        np.clip(mu * MUS, -240.0, 240.0).astype(F8))
    W1b = np.ascontiguousarray(W1.astype(BF))
    W2b = np.ascontiguousarray(W2.astype(BF))

    shared = dict(gamma=gamma, beta=beta, mu=muq, scale=scale, W1=W1b, b1=b1,
                  W2=W2b, b2=b2)
    in_maps = [dict(x=xb[b], **shared) for b in range(n_cores)]
    import os
    trace = bool(os.environ.get("SOFTMOE_TRACE"))
    res = run_bass_kernel_spmd(nc, in_maps, core_ids=list(range(n_cores)),
                               trace=trace)
    global LAST_RESULT
    LAST_RESULT = res
    return np.stack([r["out"] for r in res.results], axis=0)


LAST_RESULT = None
